# revision 16
# baseline (speedup 1.0000x reference)
"""Multi-head attention Trainium2 kernel (8 NeuronCores, SPMD).

Problem: B=4, T=2048, n_feat=512, H=8 heads, d_k=64.
Sharding: core c -> batch b = c//2, head-half hh = c%2 (4 heads = 256 attn dims).
Each core computes, for its (b, head-half):
    Q^T/K^T projections in [o, t] layout, V in [t, o] layout (+ ones column),
    flash-style attention with scores transposed (S^T[j, i]) so the softmax
    denominator comes out of the PV matmul for free, then the partial output
    projection out^T = Wo_blk @ x^T in [o2, t] layout.
Host sums the two head-half partials per batch, transposes, adds bo.

Matmuls run in float32r (TF32-like, ~1e-4 rel err, full PE rate); exp on ACT.
"""
import sys

sys.path.insert(0, "/opt/trn_rl_repo")

import numpy as np

import concourse.bass as bass
import concourse.tile as tile
from concourse import bacc, mybir
from concourse.bass_utils import run_bass_kernel_spmd

P = 128
T = 2048
F = 512            # n_feat (projection contraction dim)
OB = 256           # per-core attention dims (4 heads x 64)
NH = 4             # local heads
DK = 64
NT = T // P        # 16 row tiles
FO = F // P        # 4 feature tiles
NSUP = 2           # i-supers per head
ISUP = T // NSUP   # 1024
NC_ = ISUP // P    # 8 chunks per super
JT = NT            # 16 j tiles
NEG = -1.0e30
EPS = 1e-8

f32 = mybir.dt.float32
f32r = mybir.dt.float32r

_CACHE = {}


def _build():
    nc = bacc.Bacc("TRN2", target_bir_lowering=False, debug=False, num_devices=8)

    xq = nc.dram_tensor("xq", (T, F), f32, kind="ExternalInput").ap()
    xk = nc.dram_tensor("xk", (T, F), f32, kind="ExternalInput").ap()
    xv = nc.dram_tensor("xv", (T, F), f32, kind="ExternalInput").ap()
    wq = nc.dram_tensor("wq", (OB, F), f32, kind="ExternalInput").ap()
    wk = nc.dram_tensor("wk", (OB, F), f32, kind="ExternalInput").ap()
    wv = nc.dram_tensor("wv", (OB, F), f32, kind="ExternalInput").ap()
    wo = nc.dram_tensor("wo", (F, OB), f32, kind="ExternalInput").ap()
    bqr = nc.dram_tensor("bqr", (P, OB // P), f32, kind="ExternalInput").ap()
    bkr = nc.dram_tensor("bkr", (P, OB // P), f32, kind="ExternalInput").ap()
    bvb = nc.dram_tensor("bvb", (P, OB), f32, kind="ExternalInput").ap()
    mb = nc.dram_tensor("mb", (P, JT), f32, kind="ExternalInput").ap()
    ident = nc.dram_tensor("ident", (P, P), f32, kind="ExternalInput").ap()
    outT = nc.dram_tensor("outT", (F, T), f32, kind="ExternalOutput").ap()

    with tile.TileContext(nc) as tc:
        with tc.tile_pool(name="const", bufs=1) as cpool, \
             tc.tile_pool(name="persist", bufs=1) as ppool, \
             tc.tile_pool(name="win", bufs=2) as wpool, \
             tc.tile_pool(name="inp", bufs=1) as ipool, \
             tc.tile_pool(name="stage", bufs=2) as spool, \
             tc.tile_pool(name="et", bufs=3) as epool, \
             tc.tile_pool(name="norm", bufs=2) as npool, \
             tc.tile_pool(name="ps", bufs=2, space="PSUM") as ps:

            def big_ps(name):
                # "big" tag: 2-bank slots shared by S^T / proj / dance tiles
                return ps.tile([P, ISUP], f32, tag="big", name=name)

            def xp_ps(name):
                # "xp" tag: 2-bank slots shared by PV accum / transpose staging
                return ps.tile([P, ISUP], f32, tag="xp", name=name)

            # ---- constants ----
            id_sb = cpool.tile([P, P], f32, tag="ident")
            nc.sync.dma_start(out=id_sb[:], in_=ident[:])
            bq_sb = cpool.tile([P, OB // P], f32, tag="bq")
            nc.sync.dma_start(out=bq_sb[:], in_=bqr[:])
            bk_sb = cpool.tile([P, OB // P], f32, tag="bk")
            nc.sync.dma_start(out=bk_sb[:], in_=bkr[:])
            bv_sb = cpool.tile([P, OB], f32, tag="bv")
            nc.sync.dma_start(out=bv_sb[:], in_=bvb[:])
            mb_sb = cpool.tile([P, JT], f32, tag="mb")
            nc.sync.dma_start(out=mb_sb[:], in_=mb[:])

            # ---- weight transpose helpers (emitted per-tensor below) ----
            wT = {}

            def emit_wT(name, wdram):
                w_sb = wpool.tile([P, OB // P, F], f32, tag="wstage")
                nc.sync.dma_start(
                    out=w_sb[:], in_=wdram.rearrange("(po p) f -> p po f", p=P)
                )
                wt = cpool.tile([P, FO, OB], f32r, tag=f"w{name}T")
                for fo in range(FO):
                    tp = xp_ps(f"wtr_{name}_{fo}")
                    for po in range(OB // P):
                        nc.tensor.transpose(
                            tp[:, po * P:(po + 1) * P],
                            w_sb[:, po, fo * P:(fo + 1) * P],
                            id_sb[:],
                        )
                    nc.scalar.copy(wt[:, fo, :], tp[:, :OB])
                wT[name] = wt

            def emit_woT():
                wo_sb = wpool.tile([P, FO, OB], f32, tag="wstage")
                nc.sync.dma_start(
                    out=wo_sb[:], in_=wo.rearrange("(a p) o -> p a o", p=P)
                )
                woT = cpool.tile([DK, NH, F], f32r, tag="woT")
                for h in range(NH):
                    tp = xp_ps(f"wotr_{h}")
                    for a in range(FO):
                        nc.tensor.transpose(
                            tp[:DK, a * P:(a + 1) * P],
                            wo_sb[:, a, h * DK:(h + 1) * DK],
                            id_sb[:],
                        )
                    nc.scalar.copy(woT[:, h, :], tp[:DK, :F])
                return woT

            # ---- persistent activations ----
            QT = ppool.tile([P, OB // P, T], f32r, tag="QT")
            KT = ppool.tile([P, OB // P, T], f32r, tag="KT")
            xT = ppool.tile([DK, NH, T], f32r, tag="xT")
            V2 = ppool.tile([P, NT, NH, DK + 1], f32r, tag="V2")
            one_sb = cpool.tile([P, NT * NH], f32, tag="ones")
            nc.vector.memset(one_sb[:], 1.0)
            nc.vector.tensor_copy(
                V2[:, :, :, DK:DK + 1],
                one_sb[:].rearrange("p (t h) -> p t h ()", t=NT),
            )

            # ---- phase 1: input transpose + projections ----
            def load_transposed(xdram, name):
                """x [T, F] -> inT [P, FO, T] f32r (partition = f%128)."""
                inT = ipool.tile([P, FO, T], f32r, tag="inT")
                xr = xdram.rearrange("(t p) f -> p t f", p=P)
                for g in range(NT // 2):
                    xs = spool.tile([P, 2, F], f32, tag="xs")
                    dma_eng = nc.sync if g % 2 == 0 else nc.gpsimd
                    dma_eng.dma_start(out=xs[:], in_=xr[:, 2 * g:2 * (g + 1), :])
                    for i in range(2):
                        t = 2 * g + i
                        tp = xp_ps(f"itr_{name}_{t}")
                        for fo in range(FO):
                            nc.tensor.transpose(
                                tp[:, fo * P:(fo + 1) * P],
                                xs[:, i, fo * P:(fo + 1) * P],
                                id_sb[:],
                            )
                        eng = nc.scalar if t % 2 == 0 else nc.vector
                        if eng is nc.scalar:
                            eng.copy(
                                inT[:, :, t * P:(t + 1) * P],
                                tp[:, :F].rearrange("p (fo q) -> p fo q", fo=FO),
                            )
                        else:
                            eng.tensor_copy(
                                inT[:, :, t * P:(t + 1) * P],
                                tp[:, :F].rearrange("p (fo q) -> p fo q", fo=FO),
                            )
                return inT

            def emit_qk_proj(name, bias_sb, dst, inT, po):
                for c in range(T // F):
                    pp = big_ps(f"proj_{name}_{po}_{c}")
                    for fo in range(FO):
                        nc.tensor.matmul(
                            pp[:, :F],
                            wT[name][:, fo, po * P:(po + 1) * P],
                            inT[:, fo, c * F:(c + 1) * F],
                            start=(fo == 0),
                            stop=(fo == FO - 1),
                        )
                    nc.vector.tensor_scalar_add(
                        dst[:, po, c * F:(c + 1) * F],
                        pp[:, :F],
                        bias_sb[:, po:po + 1],
                    )

            # K first (scores need all of KT po=0), then Q, then V
            emit_wT("k", wk)
            inT_k = load_transposed(xk, "k")
            emit_qk_proj("k", bk_sb, KT, inT_k, 0)
            emit_qk_proj("k", bk_sb, KT, inT_k, 1)
            emit_wT("q", wq)
            inT_q = load_transposed(xq, "q")
            emit_qk_proj("q", bq_sb, QT, inT_q, 0)
            emit_qk_proj("q", bq_sb, QT, inT_q, 1)

            # V: natural [t, o] layout + bias, interleaved into V2
            emit_wT("v", wv)
            inT_v = load_transposed(xv, "v")
            for t in range(NT):
                pp = big_ps(f"proj_v_{t}")
                for fo in range(FO):
                    nc.tensor.matmul(
                        pp[:, :OB],
                        inT_v[:, fo, t * P:(t + 1) * P],
                        wT["v"][:, fo, :],
                        start=(fo == 0),
                        stop=(fo == FO - 1),
                    )
                nc.vector.tensor_add(
                    V2[:, t, :, 0:DK],
                    pp[:, :OB].rearrange("p (h d) -> p h d", h=NH),
                    bv_sb[:].rearrange("p (h d) -> p h d", h=NH),
                )
            # ---- phase 2: attention ----
            def emit_jloop(h, su, dance_cb=None, jt_cb=None):
                qoff = (h % 2) * DK
                qpo = h // 2
                isl = su * ISUP
                xp = xp_ps(f"xp_{h}_{su}")

                def scores(jt):
                    st = big_ps(f"st_{h}_{su}_{jt}")
                    for c in range(ISUP // F):
                        nc.tensor.matmul(
                            st[:, c * F:(c + 1) * F],
                            KT[qoff:qoff + DK, qpo, jt * P:(jt + 1) * P],
                            QT[qoff:qoff + DK, qpo, isl + c * F:isl + (c + 1) * F],
                            start=True,
                            stop=True,
                        )
                    return st

                st_prev = scores(0)
                for jt in range(JT):
                    et = epool.tile([P, ISUP], f32r, tag="et")
                    nc.scalar.activation(
                        et[:],
                        st_prev[:],
                        mybir.ActivationFunctionType.Exp,
                        bias=mb_sb[:, jt:jt + 1],
                        scale=0.125,
                    )
                    if jt + 1 < JT:
                        st_prev = scores(jt + 1)
                    for c in range(ISUP // F):
                        nc.tensor.matmul(
                            xp[:DK + 1, c * F:(c + 1) * F],
                            V2[:, jt, h, :],
                            et[:, c * F:(c + 1) * F],
                            start=(jt == 0),
                            stop=(jt == JT - 1),
                        )
                    if jt == 3 and dance_cb is not None:
                        dance_cb()
                    if jt_cb is not None:
                        jt_cb(jt)
                return xp

            def emit_norm(h, su, xp):
                isl = su * ISUP
                # Z row (partition DK of xp psum) -> SBUF
                zst = npool.tile([1, ISUP], f32, tag="zrow")
                nc.vector.tensor_copy(zst[:], xp[DK:DK + 1, :ISUP])
                # transpose Z chunks onto partitions: zcol [P, NC_]
                zcol = big_ps(f"zcol_{h}_{su}")
                for c in range(NC_):
                    nc.tensor.transpose(
                        zcol[:, c:c + 1],
                        zst[:, c * P:(c + 1) * P],
                        id_sb[0:1, 0:1],
                    )
                # r = 1 / (Z + eps), partition-parallel
                zeps = npool.tile([P, NC_], f32, tag="zeps")
                nc.vector.tensor_scalar_add(zeps[:], zcol[:, :NC_], EPS)
                rcol = npool.tile([P, NC_], f32, tag="rcol")
                nc.vector.reciprocal(rcol[:], zeps[:])
                # transpose back: rT_ps [NC_, P] (row c holds r[c*128 : (c+1)*128])
                rT_ps = big_ps(f"rT_{h}_{su}")
                nc.tensor.transpose(rT_ps[:NC_, :P], rcol[:], id_sb[:])
                rT_sb = npool.tile([NC_, P], f32, tag="rT_sb")
                nc.vector.tensor_copy(rT_sb[:], rT_ps[:NC_, :P])
                # gather rows into one [1, ISUP] SBUF row (partition shift via DMA)
                rrow = npool.tile([1, ISUP], f32, tag="zrow")
                for c in range(NC_):
                    nc.sync.dma_start(
                        out=rrow[:, c * P:(c + 1) * P],
                        in_=rT_sb[c:c + 1, :P],
                    )
                # broadcast across DK partitions (gpsimd)
                rb = npool.tile([DK, ISUP], f32, tag="rb")
                nc.gpsimd.partition_broadcast(rb[:], rrow[:])
                # x^T = x'^T * r
                nc.vector.tensor_mul(
                    xT[:, h, isl:isl + ISUP],
                    xp[0:DK, :ISUP],
                    rb[:],
                )

            woT = emit_woT()

            def emit_outproj(m2, half, psf):
                os2 = spool.tile([P, 2, F], f32, tag="os2")
                for cc in range(2):
                    c = 2 * half + cc
                    pp = psf(f"op_{m2}_{c}")
                    for h in range(NH):
                        nc.tensor.matmul(
                            pp[:, :F],
                            woT[:, h, m2 * P:(m2 + 1) * P],
                            xT[:, h, c * F:(c + 1) * F],
                            start=(h == 0),
                            stop=(h == NH - 1),
                        )
                    nc.vector.tensor_copy(os2[:, cc, :], pp[:, :F])
                nc.sync.dma_start(
                    out=outT[m2 * P:(m2 + 1) * P, half * 2 * F:(half + 1) * 2 * F],
                    in_=os2[:].rearrange("p c f -> p (c f)"),
                )

            pairs = [(h, su) for h in range(NH) for su in range(NSUP)]
            pending = [None]

            def dance_cb():
                if pending[0] is not None:
                    emit_norm(*pending[0])
                    pending[0] = None

            for idx, (h, su) in enumerate(pairs):
                if idx + 1 == len(pairs):
                    def late_cb(jt):
                        if jt == 3:
                            dance_cb()
                        elif jt in (6, 8, 10, 12):
                            emit_outproj((jt - 6) // 2, 0, xp_ps)
                    xp = emit_jloop(h, su, None, late_cb)
                else:
                    xp = emit_jloop(h, su, dance_cb)
                pending[0] = (h, su, xp)
            emit_norm(*pending[0])

            # ---- phase 3: remaining output projection (columns su=1) ----
            for m2 in range(F // P):
                emit_outproj(m2, 1, big_ps)

    nc.compile()
    return nc


def _prep_in_maps(query, key, value, mask, Wq, bq, Wk, bk, Wv, bv, Wo):
    ident = np.eye(P, dtype=np.float32)
    in_maps = []
    for c in range(8):
        b = c // 2
        hh = c % 2
        ob = slice(hh * OB, (hh + 1) * OB)
        mbias = np.where(mask[b, 0, :] == 0, np.float32(NEG), np.float32(0.0))
        mbias = np.ascontiguousarray(mbias.reshape(JT, P).T)
        in_maps.append({
            "xq": np.ascontiguousarray(query[b]),
            "xk": np.ascontiguousarray(key[b]),
            "xv": np.ascontiguousarray(value[b]),
            "wq": np.ascontiguousarray(Wq[ob, :]),
            "wk": np.ascontiguousarray(Wk[ob, :]),
            "wv": np.ascontiguousarray(Wv[ob, :]),
            "wo": np.ascontiguousarray(Wo[:, ob]),
            "bqr": np.ascontiguousarray(bq[ob].reshape(OB // P, P).T),
            "bkr": np.ascontiguousarray(bk[ob].reshape(OB // P, P).T),
            "bvb": np.ascontiguousarray(np.tile(bv[ob][None, :], (P, 1))),
            "mb": mbias,
            "ident": ident,
        })
    return in_maps


def kernel(query, key, value, mask, Wq, bq, Wk, bk, Wv, bv, Wo, bo):
    query = np.asarray(query, dtype=np.float32)
    key = np.asarray(key, dtype=np.float32)
    value = np.asarray(value, dtype=np.float32)
    mask = np.asarray(mask)
    Wq = np.asarray(Wq, dtype=np.float32)
    bq = np.asarray(bq, dtype=np.float32)
    Wk = np.asarray(Wk, dtype=np.float32)
    bk = np.asarray(bk, dtype=np.float32)
    Wv = np.asarray(Wv, dtype=np.float32)
    bv = np.asarray(bv, dtype=np.float32)
    Wo = np.asarray(Wo, dtype=np.float32)
    bo = np.asarray(bo, dtype=np.float32)

    if "nc" not in _CACHE:
        _CACHE["nc"] = _build()
    nc = _CACHE["nc"]

    B = query.shape[0]
    in_maps = _prep_in_maps(query, key, value, mask, Wq, bq, Wk, bk, Wv, bv, Wo)
    res = run_bass_kernel_spmd(nc, in_maps, core_ids=list(range(8)))

    out = np.empty((B, T, F), dtype=np.float32)
    for b in range(B):
        acc = res.results[2 * b]["outT"] + res.results[2 * b + 1]["outT"]
        out[b] = acc.T + bo[None, :]
    return out


# revision 21
# speedup vs baseline: 1.0769x; 1.0769x over previous
"""Multi-head attention Trainium2 kernel (8 NeuronCores, SPMD).

Problem: B=4, T=2048, n_feat=512, H=8 heads, d_k=64.
Sharding: core c -> batch b = c//2, head-half hh = c%2 (4 heads = 256 attn dims).
Each core computes, for its (b, head-half):
    Q^T/K^T projections in [o, t] layout, V in [t, o] layout (+ ones column),
    flash-style attention with scores transposed (S^T[j, i]) so the softmax
    denominator comes out of the PV matmul for free, then the partial output
    projection out^T = Wo_blk @ x^T in [o2, t] layout.
Host sums the two head-half partials per batch, transposes, adds bo.

Matmuls run in float32r (TF32-like, ~1e-4 rel err, full PE rate); exp on ACT.
"""
import sys

sys.path.insert(0, "/opt/trn_rl_repo")

import numpy as np

import concourse.bass as bass
import concourse.tile as tile
from concourse import bacc, mybir
from concourse.bass_utils import run_bass_kernel_spmd

P = 128
T = 2048
F = 512            # n_feat (projection contraction dim)
OB = 256           # per-core attention dims (4 heads x 64)
NH = 4             # local heads
DK = 64
NT = T // P        # 16 row tiles
FO = F // P        # 4 feature tiles
NSUP = 2           # i-supers per head
ISUP = T // NSUP   # 1024
NC_ = ISUP // P    # 8 chunks per super
JT = NT            # 16 j tiles
NEG = -1.0e30
EPS = 1e-8

f32 = mybir.dt.float32
f32r = mybir.dt.float32r

_CACHE = {}


def _build():
    nc = bacc.Bacc("TRN2", target_bir_lowering=False, debug=False, num_devices=8)

    xq = nc.dram_tensor("xq", (T, F), f32, kind="ExternalInput").ap()
    xk = nc.dram_tensor("xk", (T, F), f32, kind="ExternalInput").ap()
    xv = nc.dram_tensor("xv", (T, F), f32, kind="ExternalInput").ap()
    wq = nc.dram_tensor("wq", (OB, F), f32, kind="ExternalInput").ap()
    wk = nc.dram_tensor("wk", (OB, F), f32, kind="ExternalInput").ap()
    wv = nc.dram_tensor("wv", (OB, F), f32, kind="ExternalInput").ap()
    wo = nc.dram_tensor("wo", (F, OB), f32, kind="ExternalInput").ap()
    bqr = nc.dram_tensor("bqr", (P, OB // P), f32, kind="ExternalInput").ap()
    bkr = nc.dram_tensor("bkr", (P, OB // P), f32, kind="ExternalInput").ap()
    bvb = nc.dram_tensor("bvb", (P, OB), f32, kind="ExternalInput").ap()
    mb = nc.dram_tensor("mb", (P, JT), f32, kind="ExternalInput").ap()
    ident = nc.dram_tensor("ident", (P, P), f32, kind="ExternalInput").ap()
    outT = nc.dram_tensor("outT", (F, T), f32, kind="ExternalOutput").ap()

    with tile.TileContext(nc) as tc:
        with tc.tile_pool(name="const", bufs=1) as cpool, \
             tc.tile_pool(name="persist", bufs=1) as ppool, \
             tc.tile_pool(name="win", bufs=2) as wpool, \
             tc.tile_pool(name="inp", bufs=1) as ipool, \
             tc.tile_pool(name="stage", bufs=4) as spool, \
             tc.tile_pool(name="et", bufs=3) as epool, \
             tc.tile_pool(name="norm", bufs=2) as npool, \
             tc.tile_pool(name="ps", bufs=2, space="PSUM") as ps:

            def big_ps(name):
                # "big" tag: 2-bank slots shared by S^T / proj / dance tiles
                return ps.tile([P, ISUP], f32, tag="big", name=name)

            def xp_ps(name):
                # "xp" tag: 2-bank slots shared by PV accum / transpose staging
                return ps.tile([P, ISUP], f32, tag="xp", name=name)

            # ---- constants ----
            id_sb = cpool.tile([P, P], f32, tag="ident")
            nc.sync.dma_start(out=id_sb[:], in_=ident[:])
            bq_sb = cpool.tile([P, OB // P], f32, tag="bq")
            nc.gpsimd.dma_start(out=bq_sb[:], in_=bqr[:])
            bk_sb = cpool.tile([P, OB // P], f32, tag="bk")
            nc.gpsimd.dma_start(out=bk_sb[:], in_=bkr[:])
            bv_sb = cpool.tile([P, OB], f32, tag="bv")
            nc.gpsimd.dma_start(out=bv_sb[:], in_=bvb[:])
            mb_sb = cpool.tile([P, JT], f32, tag="mb")
            nc.gpsimd.dma_start(out=mb_sb[:], in_=mb[:])

            # ---- weight transpose helpers (emitted per-tensor below) ----
            wT = {}

            def emit_wT(name, wdram):
                w_sb = wpool.tile([P, OB // P, F], f32, tag="wstage")
                nc.gpsimd.dma_start(
                    out=w_sb[:], in_=wdram.rearrange("(po p) f -> p po f", p=P)
                )
                wt = cpool.tile([P, FO, OB], f32r, tag=f"w{name}T")
                for fo in range(FO):
                    tp = xp_ps(f"wtr_{name}_{fo}")
                    for po in range(OB // P):
                        nc.tensor.transpose(
                            tp[:, po * P:(po + 1) * P],
                            w_sb[:, po, fo * P:(fo + 1) * P],
                            id_sb[:],
                        )
                    nc.scalar.copy(wt[:, fo, :], tp[:, :OB])
                wT[name] = wt

            def emit_woT():
                wo_sb = wpool.tile([P, FO, OB], f32, tag="wstage")
                nc.sync.dma_start(
                    out=wo_sb[:], in_=wo.rearrange("(a p) o -> p a o", p=P)
                )
                woT = cpool.tile([DK, NH, F], f32r, tag="woT")
                for h in range(NH):
                    tp = xp_ps(f"wotr_{h}")
                    for a in range(FO):
                        nc.tensor.transpose(
                            tp[:DK, a * P:(a + 1) * P],
                            wo_sb[:, a, h * DK:(h + 1) * DK],
                            id_sb[:],
                        )
                    nc.scalar.copy(woT[:, h, :], tp[:DK, :F])
                return woT

            # ---- persistent activations ----
            QT = ppool.tile([P, OB // P, T], f32r, tag="QT")
            KT = ppool.tile([P, OB // P, T], f32r, tag="KT")
            xT = ppool.tile([DK, NH, T], f32r, tag="xT")
            V2 = ppool.tile([P, NT, NH, DK + 1], f32r, tag="V2")
            one_sb = cpool.tile([P, NT * NH], f32, tag="ones")
            nc.vector.memset(one_sb[:], 1.0)
            nc.vector.tensor_copy(
                V2[:, :, :, DK:DK + 1],
                one_sb[:].rearrange("p (t h) -> p t h ()", t=NT),
            )

            # ---- phase 1: input transpose + projections ----
            def load_transposed(xdram, name):
                """x [T, F] -> inT [P, FO, T] f32r (partition = f%128)."""
                inT = ipool.tile([P, FO, T], f32r, tag="inT")
                xr = xdram.rearrange("(t p) f -> p t f", p=P)
                for g in range(NT // 2):
                    xs = spool.tile([P, 2, F], f32, tag="xs")
                    dma_eng = nc.sync if g % 2 == 0 else nc.gpsimd
                    dma_eng.dma_start(out=xs[:], in_=xr[:, 2 * g:2 * (g + 1), :])
                    t = 2 * g
                    tp = xp_ps(f"itr_{name}_{t}")
                    for i in range(2):
                        for fo in range(FO):
                            nc.tensor.transpose(
                                tp[:, i * F + fo * P:i * F + (fo + 1) * P],
                                xs[:, i, fo * P:(fo + 1) * P],
                                id_sb[:],
                            )
                    src_ap = tp[:, :2 * F].rearrange(
                        "p (i fo q) -> p fo i q", i=2, fo=FO
                    )
                    dst_ap = inT[:, :, t * P:(t + 2) * P].rearrange(
                        "p fo (i q) -> p fo i q", i=2
                    )
                    if g % 2 == 0:
                        nc.scalar.copy(dst_ap, src_ap)
                    else:
                        nc.vector.tensor_copy(dst_ap, src_ap)
                return inT

            def emit_qk_proj(name, bias_sb, dst, inT, po):
                for c in range(T // F):
                    pp = big_ps(f"proj_{name}_{po}_{c}")
                    for fo in range(FO):
                        nc.tensor.matmul(
                            pp[:, :F],
                            wT[name][:, fo, po * P:(po + 1) * P],
                            inT[:, fo, c * F:(c + 1) * F],
                            start=(fo == 0),
                            stop=(fo == FO - 1),
                        )
                    nc.vector.tensor_scalar_add(
                        dst[:, po, c * F:(c + 1) * F],
                        pp[:, :F],
                        bias_sb[:, po:po + 1],
                    )

            # K first (scores need all of KT po=0), then Q, then V
            emit_wT("k", wk)
            inT_k = load_transposed(xk, "k")
            emit_qk_proj("k", bk_sb, KT, inT_k, 0)
            emit_qk_proj("k", bk_sb, KT, inT_k, 1)
            emit_wT("q", wq)
            inT_q = load_transposed(xq, "q")
            emit_qk_proj("q", bq_sb, QT, inT_q, 0)
            emit_qk_proj("q", bq_sb, QT, inT_q, 1)

            # V: natural [t, o] layout + bias, interleaved into V2
            emit_wT("v", wv)
            inT_v = load_transposed(xv, "v")
            for t in range(NT):
                pp = big_ps(f"proj_v_{t}")
                for fo in range(FO):
                    nc.tensor.matmul(
                        pp[:, :OB],
                        inT_v[:, fo, t * P:(t + 1) * P],
                        wT["v"][:, fo, :],
                        start=(fo == 0),
                        stop=(fo == FO - 1),
                    )
                nc.vector.tensor_add(
                    V2[:, t, :, 0:DK],
                    pp[:, :OB].rearrange("p (h d) -> p h d", h=NH),
                    bv_sb[:].rearrange("p (h d) -> p h d", h=NH),
                )
            # ---- phase 2: attention ----
            def emit_jloop(h, su, dance_cb=None, jt_cb=None):
                qoff = (h % 2) * DK
                qpo = h // 2
                isl = su * ISUP
                xp = xp_ps(f"xp_{h}_{su}")

                def scores(jt):
                    st = big_ps(f"st_{h}_{su}_{jt}")
                    for c in range(ISUP // F):
                        nc.tensor.matmul(
                            st[:, c * F:(c + 1) * F],
                            KT[qoff:qoff + DK, qpo, jt * P:(jt + 1) * P],
                            QT[qoff:qoff + DK, qpo, isl + c * F:isl + (c + 1) * F],
                            start=True,
                            stop=True,
                        )
                    return st

                st_prev = scores(0)
                for jt in range(JT):
                    et = epool.tile([P, ISUP], f32r, tag="et")
                    nc.scalar.activation(
                        et[:],
                        st_prev[:],
                        mybir.ActivationFunctionType.Exp,
                        bias=mb_sb[:, jt:jt + 1],
                        scale=0.125,
                    )
                    if jt + 1 < JT:
                        st_prev = scores(jt + 1)
                    for c in range(ISUP // F):
                        nc.tensor.matmul(
                            xp[:DK + 1, c * F:(c + 1) * F],
                            V2[:, jt, h, :],
                            et[:, c * F:(c + 1) * F],
                            start=(jt == 0),
                            stop=(jt == JT - 1),
                        )
                    if jt == 3 and dance_cb is not None:
                        dance_cb()
                    if jt_cb is not None:
                        jt_cb(jt)
                return xp

            def emit_norm(h, su, xp):
                isl = su * ISUP
                # Z row (partition DK of xp psum) -> SBUF
                zst = npool.tile([1, ISUP], f32, tag="zrow")
                nc.vector.tensor_copy(zst[:], xp[DK:DK + 1, :ISUP])
                # transpose Z chunks onto partitions: zcol [P, NC_]
                zcol = big_ps(f"zcol_{h}_{su}")
                for c in range(NC_):
                    nc.tensor.transpose(
                        zcol[:, c:c + 1],
                        zst[:, c * P:(c + 1) * P],
                        id_sb[0:1, 0:1],
                    )
                # r = 1 / (Z + eps), partition-parallel
                zeps = npool.tile([P, NC_], f32, tag="zeps")
                nc.vector.tensor_scalar_add(zeps[:], zcol[:, :NC_], EPS)
                rcol = npool.tile([P, NC_], f32, tag="rcol")
                nc.vector.reciprocal(rcol[:], zeps[:])
                # transpose back: rT_ps [NC_, P] (row c holds r[c*128 : (c+1)*128])
                rT_ps = big_ps(f"rT_{h}_{su}")
                nc.tensor.transpose(rT_ps[:NC_, :P], rcol[:], id_sb[:])
                rT_sb = npool.tile([NC_, P], f32, tag="rT_sb")
                nc.vector.tensor_copy(rT_sb[:], rT_ps[:NC_, :P])
                # gather rows into one [1, ISUP] SBUF row (partition shift via DMA)
                rrow = npool.tile([1, ISUP], f32, tag="zrow")
                for c in range(NC_):
                    (nc.sync if c % 2 == 0 else nc.gpsimd).dma_start(
                        out=rrow[:, c * P:(c + 1) * P],
                        in_=rT_sb[c:c + 1, :P],
                    )
                # broadcast across DK partitions (gpsimd)
                rb = npool.tile([DK, ISUP], f32, tag="rb")
                nc.gpsimd.partition_broadcast(rb[:], rrow[:])
                # x^T = x'^T * r
                nc.vector.tensor_mul(
                    xT[:, h, isl:isl + ISUP],
                    xp[0:DK, :ISUP],
                    rb[:],
                )

            woT = emit_woT()

            def emit_outproj(m2, half, psf):
                os2 = spool.tile([P, 2, F], f32, tag="os2")
                for cc in range(2):
                    c = 2 * half + cc
                    pp = psf(f"op_{m2}_{c}")
                    for h in range(NH):
                        nc.tensor.matmul(
                            pp[:, :F],
                            woT[:, h, m2 * P:(m2 + 1) * P],
                            xT[:, h, c * F:(c + 1) * F],
                            start=(h == 0),
                            stop=(h == NH - 1),
                        )
                    nc.vector.tensor_copy(os2[:, cc, :], pp[:, :F])
                nc.sync.dma_start(
                    out=outT[m2 * P:(m2 + 1) * P, half * 2 * F:(half + 1) * 2 * F],
                    in_=os2[:].rearrange("p c f -> p (c f)"),
                )

            pairs = [(h, su) for h in range(NH) for su in range(NSUP)]
            pending = [None]

            def dance_cb():
                if pending[0] is not None:
                    emit_norm(*pending[0])
                    pending[0] = None

            for idx, (h, su) in enumerate(pairs):
                if idx + 1 == len(pairs):
                    def late_cb(jt):
                        if jt == 3:
                            dance_cb()
                        elif jt in (6, 8, 10, 12):
                            emit_outproj((jt - 6) // 2, 0, xp_ps)
                    xp = emit_jloop(h, su, None, late_cb)
                else:
                    xp = emit_jloop(h, su, dance_cb)
                pending[0] = (h, su, xp)
            emit_norm(*pending[0])

            # ---- phase 3: remaining output projection (columns su=1) ----
            for m2 in range(F // P):
                emit_outproj(m2, 1, big_ps)

    nc.compile()
    return nc


def _prep_in_maps(query, key, value, mask, Wq, bq, Wk, bk, Wv, bv, Wo):
    ident = np.eye(P, dtype=np.float32)
    in_maps = []
    for c in range(8):
        b = c // 2
        hh = c % 2
        ob = slice(hh * OB, (hh + 1) * OB)
        mbias = np.where(mask[b, 0, :] == 0, np.float32(NEG), np.float32(0.0))
        mbias = np.ascontiguousarray(mbias.reshape(JT, P).T)
        in_maps.append({
            "xq": np.ascontiguousarray(query[b]),
            "xk": np.ascontiguousarray(key[b]),
            "xv": np.ascontiguousarray(value[b]),
            "wq": np.ascontiguousarray(Wq[ob, :]),
            "wk": np.ascontiguousarray(Wk[ob, :]),
            "wv": np.ascontiguousarray(Wv[ob, :]),
            "wo": np.ascontiguousarray(Wo[:, ob]),
            "bqr": np.ascontiguousarray(bq[ob].reshape(OB // P, P).T),
            "bkr": np.ascontiguousarray(bk[ob].reshape(OB // P, P).T),
            "bvb": np.ascontiguousarray(np.tile(bv[ob][None, :], (P, 1))),
            "mb": mbias,
            "ident": ident,
        })
    return in_maps


def kernel(query, key, value, mask, Wq, bq, Wk, bk, Wv, bv, Wo, bo):
    query = np.asarray(query, dtype=np.float32)
    key = np.asarray(key, dtype=np.float32)
    value = np.asarray(value, dtype=np.float32)
    mask = np.asarray(mask)
    Wq = np.asarray(Wq, dtype=np.float32)
    bq = np.asarray(bq, dtype=np.float32)
    Wk = np.asarray(Wk, dtype=np.float32)
    bk = np.asarray(bk, dtype=np.float32)
    Wv = np.asarray(Wv, dtype=np.float32)
    bv = np.asarray(bv, dtype=np.float32)
    Wo = np.asarray(Wo, dtype=np.float32)
    bo = np.asarray(bo, dtype=np.float32)

    if "nc" not in _CACHE:
        _CACHE["nc"] = _build()
    nc = _CACHE["nc"]

    B = query.shape[0]
    in_maps = _prep_in_maps(query, key, value, mask, Wq, bq, Wk, bk, Wv, bv, Wo)
    res = run_bass_kernel_spmd(nc, in_maps, core_ids=list(range(8)))

    out = np.empty((B, T, F), dtype=np.float32)
    for b in range(B):
        acc = res.results[2 * b]["outT"] + res.results[2 * b + 1]["outT"]
        out[b] = acc.T + bo[None, :]
    return out


# revision 23
# speedup vs baseline: 1.0775x; 1.0006x over previous
"""Multi-head attention Trainium2 kernel (8 NeuronCores, SPMD).

Problem: B=4, T=2048, n_feat=512, H=8 heads, d_k=64.
Sharding: core c -> batch b = c//2, head-half hh = c%2 (4 heads = 256 attn dims).
Each core computes, for its (b, head-half):
    Q^T/K^T projections in [o, t] layout, V in [t, o] layout (+ ones column),
    flash-style attention with scores transposed (S^T[j, i]) so the softmax
    denominator comes out of the PV matmul for free, then the partial output
    projection out^T = Wo_blk @ x^T in [o2, t] layout.
Host sums the two head-half partials per batch, transposes, adds bo.

Matmuls run in float32r (TF32-like, ~1e-4 rel err, full PE rate); exp on ACT.
"""
import sys

sys.path.insert(0, "/opt/trn_rl_repo")

import numpy as np

import concourse.bass as bass
import concourse.tile as tile
from concourse import bacc, mybir
from concourse.bass_utils import run_bass_kernel_spmd

P = 128
T = 2048
F = 512            # n_feat (projection contraction dim)
OB = 256           # per-core attention dims (4 heads x 64)
NH = 4             # local heads
DK = 64
NT = T // P        # 16 row tiles
FO = F // P        # 4 feature tiles
NSUP = 2           # i-supers per head
ISUP = T // NSUP   # 1024
NC_ = ISUP // P    # 8 chunks per super
JT = NT            # 16 j tiles
NEG = -1.0e30
EPS = 1e-8

f32 = mybir.dt.float32
f32r = mybir.dt.float32r

_CACHE = {}


def _build():
    nc = bacc.Bacc("TRN2", target_bir_lowering=False, debug=False, num_devices=8)

    xq = nc.dram_tensor("xq", (T, F), f32, kind="ExternalInput").ap()
    xk = nc.dram_tensor("xk", (T, F), f32, kind="ExternalInput").ap()
    xv = nc.dram_tensor("xv", (T, F), f32, kind="ExternalInput").ap()
    wq = nc.dram_tensor("wq", (OB, F), f32, kind="ExternalInput").ap()
    wk = nc.dram_tensor("wk", (OB, F), f32, kind="ExternalInput").ap()
    wv = nc.dram_tensor("wv", (OB, F), f32, kind="ExternalInput").ap()
    wo = nc.dram_tensor("wo", (F, OB), f32, kind="ExternalInput").ap()
    bqr = nc.dram_tensor("bqr", (P, OB // P), f32, kind="ExternalInput").ap()
    bkr = nc.dram_tensor("bkr", (P, OB // P), f32, kind="ExternalInput").ap()
    bvb = nc.dram_tensor("bvb", (P, OB), f32, kind="ExternalInput").ap()
    mb = nc.dram_tensor("mb", (P, JT), f32, kind="ExternalInput").ap()
    ident = nc.dram_tensor("ident", (P, P), f32, kind="ExternalInput").ap()
    outT = nc.dram_tensor("outT", (F, T), f32, kind="ExternalOutput").ap()

    with tile.TileContext(nc) as tc:
        with tc.tile_pool(name="const", bufs=1) as cpool, \
             tc.tile_pool(name="persist", bufs=1) as ppool, \
             tc.tile_pool(name="win", bufs=2) as wpool, \
             tc.tile_pool(name="inp", bufs=1) as ipool, \
             tc.tile_pool(name="stage", bufs=4) as spool, \
             tc.tile_pool(name="et", bufs=3) as epool, \
             tc.tile_pool(name="norm", bufs=2) as npool, \
             tc.tile_pool(name="ps", bufs=2, space="PSUM") as ps:

            def big_ps(name):
                # "big" tag: 2-bank slots shared by S^T / proj / dance tiles
                return ps.tile([P, ISUP], f32, tag="big", name=name)

            def xp_ps(name):
                # "xp" tag: 2-bank slots shared by PV accum / transpose staging
                return ps.tile([P, ISUP], f32, tag="xp", name=name)

            # ---- constants ----
            id_sb = cpool.tile([P, P], f32, tag="ident")
            nc.sync.dma_start(out=id_sb[:], in_=ident[:])
            bq_sb = cpool.tile([P, OB // P], f32, tag="bq")
            nc.gpsimd.dma_start(out=bq_sb[:], in_=bqr[:])
            bk_sb = cpool.tile([P, OB // P], f32, tag="bk")
            nc.gpsimd.dma_start(out=bk_sb[:], in_=bkr[:])
            bv_sb = cpool.tile([P, OB], f32, tag="bv")
            nc.gpsimd.dma_start(out=bv_sb[:], in_=bvb[:])
            mb_sb = cpool.tile([P, JT], f32, tag="mb")
            nc.gpsimd.dma_start(out=mb_sb[:], in_=mb[:])

            # ---- weight transpose helpers (emitted per-tensor below) ----
            wT = {}

            def emit_wT(name, wdram):
                w_sb = wpool.tile([P, OB // P, F], f32, tag="wstage")
                nc.gpsimd.dma_start(
                    out=w_sb[:], in_=wdram.rearrange("(po p) f -> p po f", p=P)
                )
                wt = cpool.tile([P, FO, OB], f32r, tag=f"w{name}T")
                for fo in range(FO):
                    tp = xp_ps(f"wtr_{name}_{fo}")
                    for po in range(OB // P):
                        nc.tensor.transpose(
                            tp[:, po * P:(po + 1) * P],
                            w_sb[:, po, fo * P:(fo + 1) * P],
                            id_sb[:],
                        )
                    nc.scalar.copy(wt[:, fo, :], tp[:, :OB])
                wT[name] = wt

            def emit_woT():
                wo_sb = wpool.tile([P, FO, OB], f32, tag="wstage")
                nc.sync.dma_start(
                    out=wo_sb[:], in_=wo.rearrange("(a p) o -> p a o", p=P)
                )
                woT = cpool.tile([DK, NH, F], f32r, tag="woT")
                for h in range(NH):
                    tp = xp_ps(f"wotr_{h}")
                    for a in range(FO):
                        nc.tensor.transpose(
                            tp[:DK, a * P:(a + 1) * P],
                            wo_sb[:, a, h * DK:(h + 1) * DK],
                            id_sb[:],
                        )
                    nc.scalar.copy(woT[:, h, :], tp[:DK, :F])
                return woT

            # ---- persistent activations ----
            QT = ppool.tile([P, OB // P, T], f32r, tag="QT")
            KT = ppool.tile([P, OB // P, T], f32r, tag="KT")
            xT = ppool.tile([DK, NH, T], f32r, tag="xT")
            V2 = ppool.tile([P, NT, NH, DK + 1], f32r, tag="V2")
            one_sb = cpool.tile([P, NT * NH], f32, tag="ones")
            nc.vector.memset(one_sb[:], 1.0)
            nc.vector.tensor_copy(
                V2[:, :, :, DK:DK + 1],
                one_sb[:].rearrange("p (t h) -> p t h ()", t=NT),
            )

            # ---- phase 1: input transpose + projections ----
            def load_transposed(xdram, name):
                """x [T, F] -> inT [P, FO, T] f32r (partition = f%128)."""
                inT = ipool.tile([P, FO, T], f32r, tag="inT")
                xr = xdram.rearrange("(t p) f -> p t f", p=P)
                for g in range(NT // 2):
                    xs = spool.tile([P, 2, F], f32, tag="xs")
                    dma_eng = nc.sync if g % 2 == 0 else nc.gpsimd
                    dma_eng.dma_start(out=xs[:], in_=xr[:, 2 * g:2 * (g + 1), :])
                    t = 2 * g
                    tp = xp_ps(f"itr_{name}_{t}")
                    for i in range(2):
                        for fo in range(FO):
                            nc.tensor.transpose(
                                tp[:, i * F + fo * P:i * F + (fo + 1) * P],
                                xs[:, i, fo * P:(fo + 1) * P],
                                id_sb[:],
                            )
                    src_ap = tp[:, :2 * F].rearrange(
                        "p (i fo q) -> p fo i q", i=2, fo=FO
                    )
                    dst_ap = inT[:, :, t * P:(t + 2) * P].rearrange(
                        "p fo (i q) -> p fo i q", i=2
                    )
                    if g % 2 == 0:
                        nc.scalar.copy(dst_ap, src_ap)
                    else:
                        nc.vector.tensor_copy(dst_ap, src_ap)
                return inT

            def emit_qk_proj(name, bias_sb, dst, inT, po):
                for c in range(T // F):
                    pp = big_ps(f"proj_{name}_{po}_{c}")
                    for fo in range(FO):
                        nc.tensor.matmul(
                            pp[:, :F],
                            wT[name][:, fo, po * P:(po + 1) * P],
                            inT[:, fo, c * F:(c + 1) * F],
                            start=(fo == 0),
                            stop=(fo == FO - 1),
                        )
                    nc.vector.tensor_scalar_add(
                        dst[:, po, c * F:(c + 1) * F],
                        pp[:, :F],
                        bias_sb[:, po:po + 1],
                    )

            # K first (scores need all of KT po=0), then Q, then V
            emit_wT("k", wk)
            inT_k = load_transposed(xk, "k")
            emit_qk_proj("k", bk_sb, KT, inT_k, 0)
            emit_qk_proj("k", bk_sb, KT, inT_k, 1)
            emit_wT("q", wq)
            inT_q = load_transposed(xq, "q")
            emit_qk_proj("q", bq_sb, QT, inT_q, 0)
            emit_qk_proj("q", bq_sb, QT, inT_q, 1)

            # V: natural [t, o] layout + bias, interleaved into V2
            emit_wT("v", wv)
            inT_v = load_transposed(xv, "v")
            for t in range(NT):
                pp = big_ps(f"proj_v_{t}")
                for fo in range(FO):
                    nc.tensor.matmul(
                        pp[:, :OB],
                        inT_v[:, fo, t * P:(t + 1) * P],
                        wT["v"][:, fo, :],
                        start=(fo == 0),
                        stop=(fo == FO - 1),
                    )
                nc.vector.tensor_add(
                    V2[:, t, :, 0:DK],
                    pp[:, :OB].rearrange("p (h d) -> p h d", h=NH),
                    bv_sb[:].rearrange("p (h d) -> p h d", h=NH),
                )
            # ---- phase 2: attention ----
            def emit_jloop(h, su, dance_cb=None, jt_cb=None):
                qoff = (h % 2) * DK
                qpo = h // 2
                isl = su * ISUP
                xp = xp_ps(f"xp_{h}_{su}")

                def scores(jt):
                    st = big_ps(f"st_{h}_{su}_{jt}")
                    for c in range(ISUP // F):
                        nc.tensor.matmul(
                            st[:, c * F:(c + 1) * F],
                            KT[qoff:qoff + DK, qpo, jt * P:(jt + 1) * P],
                            QT[qoff:qoff + DK, qpo, isl + c * F:isl + (c + 1) * F],
                            start=True,
                            stop=True,
                        )
                    return st

                st_prev = scores(0)
                for jt in range(JT):
                    et = epool.tile([P, ISUP], f32r, tag="et")
                    nc.scalar.activation(
                        et[:],
                        st_prev[:],
                        mybir.ActivationFunctionType.Exp,
                        bias=mb_sb[:, jt:jt + 1],
                        scale=0.125,
                    )
                    if jt + 1 < JT:
                        st_prev = scores(jt + 1)
                    for c in range(ISUP // F):
                        nc.tensor.matmul(
                            xp[:DK + 1, c * F:(c + 1) * F],
                            V2[:, jt, h, :],
                            et[:, c * F:(c + 1) * F],
                            start=(jt == 0),
                            stop=(jt == JT - 1),
                        )
                    if jt == 2 and dance_cb is not None:
                        dance_cb()
                    if jt_cb is not None:
                        jt_cb(jt)
                return xp

            def emit_norm(h, su, xp):
                isl = su * ISUP
                # Z row (partition DK of xp psum) -> SBUF
                zst = npool.tile([1, ISUP], f32, tag="zrow")
                nc.vector.tensor_copy(zst[:], xp[DK:DK + 1, :ISUP])
                # transpose Z chunks onto partitions: zcol [P, NC_]
                zcol = big_ps(f"zcol_{h}_{su}")
                for c in range(NC_):
                    nc.tensor.transpose(
                        zcol[:, c:c + 1],
                        zst[:, c * P:(c + 1) * P],
                        id_sb[0:1, 0:1],
                    )
                # r = 1 / (Z + eps), partition-parallel
                zeps = npool.tile([P, NC_], f32, tag="zeps")
                nc.vector.tensor_scalar_add(zeps[:], zcol[:, :NC_], EPS)
                rcol = npool.tile([P, NC_], f32, tag="rcol")
                nc.vector.reciprocal(rcol[:], zeps[:])
                # transpose back into spare columns of the same psum tile:
                # rT rows [NC_, P] at cols [P, 2P) (disjoint from zcol's cols)
                rT_ps = zcol[:NC_, P:2 * P]
                nc.tensor.transpose(rT_ps, rcol[:], id_sb[:])
                rT_sb = npool.tile([NC_, P], f32, tag="rT_sb")
                nc.vector.tensor_copy(rT_sb[:], rT_ps)
                # gather rows into one [1, ISUP] SBUF row (partition shift via DMA)
                rrow = npool.tile([1, ISUP], f32, tag="zrow")
                for c in range(NC_):
                    (nc.sync if c % 2 == 0 else nc.gpsimd).dma_start(
                        out=rrow[:, c * P:(c + 1) * P],
                        in_=rT_sb[c:c + 1, :P],
                    )
                # broadcast across DK partitions (gpsimd)
                rb = npool.tile([DK, ISUP], f32, tag="rb")
                nc.gpsimd.partition_broadcast(rb[:], rrow[:])
                # x^T = x'^T * r
                nc.vector.tensor_mul(
                    xT[:, h, isl:isl + ISUP],
                    xp[0:DK, :ISUP],
                    rb[:],
                )

            woT = emit_woT()

            def emit_outproj(m2, half, psf):
                os2 = spool.tile([P, 2, F], f32, tag="os2")
                for cc in range(2):
                    c = 2 * half + cc
                    pp = psf(f"op_{m2}_{c}")
                    for h in range(NH):
                        nc.tensor.matmul(
                            pp[:, :F],
                            woT[:, h, m2 * P:(m2 + 1) * P],
                            xT[:, h, c * F:(c + 1) * F],
                            start=(h == 0),
                            stop=(h == NH - 1),
                        )
                    nc.vector.tensor_copy(os2[:, cc, :], pp[:, :F])
                nc.sync.dma_start(
                    out=outT[m2 * P:(m2 + 1) * P, half * 2 * F:(half + 1) * 2 * F],
                    in_=os2[:].rearrange("p c f -> p (c f)"),
                )

            pairs = [(h, su) for h in range(NH) for su in range(NSUP)]
            pending = [None]

            def dance_cb():
                if pending[0] is not None:
                    emit_norm(*pending[0])
                    pending[0] = None

            for idx, (h, su) in enumerate(pairs):
                if idx + 1 == len(pairs):
                    def late_cb(jt):
                        if jt == 3:
                            dance_cb()
                        elif jt in (6, 8, 10, 12):
                            emit_outproj((jt - 6) // 2, 0, xp_ps)
                    xp = emit_jloop(h, su, None, late_cb)
                else:
                    xp = emit_jloop(h, su, dance_cb)
                pending[0] = (h, su, xp)
            emit_norm(*pending[0])

            # ---- phase 3: remaining output projection (columns su=1) ----
            for m2 in range(F // P):
                emit_outproj(m2, 1, big_ps)

    nc.compile()
    return nc


def _prep_in_maps(query, key, value, mask, Wq, bq, Wk, bk, Wv, bv, Wo):
    ident = np.eye(P, dtype=np.float32)
    in_maps = []
    for c in range(8):
        b = c // 2
        hh = c % 2
        ob = slice(hh * OB, (hh + 1) * OB)
        mbias = np.where(mask[b, 0, :] == 0, np.float32(NEG), np.float32(0.0))
        mbias = np.ascontiguousarray(mbias.reshape(JT, P).T)
        in_maps.append({
            "xq": np.ascontiguousarray(query[b]),
            "xk": np.ascontiguousarray(key[b]),
            "xv": np.ascontiguousarray(value[b]),
            "wq": np.ascontiguousarray(Wq[ob, :]),
            "wk": np.ascontiguousarray(Wk[ob, :]),
            "wv": np.ascontiguousarray(Wv[ob, :]),
            "wo": np.ascontiguousarray(Wo[:, ob]),
            "bqr": np.ascontiguousarray(bq[ob].reshape(OB // P, P).T),
            "bkr": np.ascontiguousarray(bk[ob].reshape(OB // P, P).T),
            "bvb": np.ascontiguousarray(np.tile(bv[ob][None, :], (P, 1))),
            "mb": mbias,
            "ident": ident,
        })
    return in_maps


def kernel(query, key, value, mask, Wq, bq, Wk, bk, Wv, bv, Wo, bo):
    query = np.asarray(query, dtype=np.float32)
    key = np.asarray(key, dtype=np.float32)
    value = np.asarray(value, dtype=np.float32)
    mask = np.asarray(mask)
    Wq = np.asarray(Wq, dtype=np.float32)
    bq = np.asarray(bq, dtype=np.float32)
    Wk = np.asarray(Wk, dtype=np.float32)
    bk = np.asarray(bk, dtype=np.float32)
    Wv = np.asarray(Wv, dtype=np.float32)
    bv = np.asarray(bv, dtype=np.float32)
    Wo = np.asarray(Wo, dtype=np.float32)
    bo = np.asarray(bo, dtype=np.float32)

    if "nc" not in _CACHE:
        _CACHE["nc"] = _build()
    nc = _CACHE["nc"]

    B = query.shape[0]
    in_maps = _prep_in_maps(query, key, value, mask, Wq, bq, Wk, bk, Wv, bv, Wo)
    res = run_bass_kernel_spmd(nc, in_maps, core_ids=list(range(8)))

    out = np.empty((B, T, F), dtype=np.float32)
    for b in range(B):
        acc = res.results[2 * b]["outT"] + res.results[2 * b + 1]["outT"]
        out[b] = acc.T + bo[None, :]
    return out
